# revision 1
# baseline (speedup 1.0000x reference)
"""Distributed GQA sliding-window attention for 8 TRN2 NeuronCores.

Sharding: tensor-parallel over heads. Core d owns query heads {2d, 2d+1} and
the single KV head d//2 they share (column-parallel wq/wk/wv, row-parallel wo).
Each core computes a full-size partial output (its heads' contribution through
its wo column slice); the host sums the 8 partials.

All device matmuls run in bf16 (fp32 PSUM accumulation). Inputs are
pre-transposed / pre-cast on the host so the kernel needs no on-device
transposes except V (16 small PE transposes). Attention processes both
query heads in one set of 512-wide matmuls (they share the KV head), and
the emission order software-pipelines Q-projection s-tile st with
attention/output-projection of s-tile st-1 so PE gaps in the attention
dependency chain are filled with projection work.

Layouts on device (partition dim first):
  xT    [HID, S]   bf16  x transposed (contraction-major for projections)
  wqT   [HID, 256] bf16  wq rows for 2 heads, transposed
  wkT   [HID, 128] bf16
  wvT   [HID, 128] bf16
  woT   [256, HID] bf16  wo columns for 2 heads, transposed
  cosT/sinPM [128, S] f32  RoPE tables (sinPM rows 0:64 negated)
  maskA/B/C/D [128, 512] f32: sliding-window boundary masks in St
        (scores-transposed) coordinates for the head-batched i-tile layout
        [h0 i-tile | h1 i-tile], extracted from the actual attention_mask
        input.
  outT  [HID, S] f32  partial output, transposed (summed + .T on host)
"""

import numpy as np
import ml_dtypes

import concourse.bass as bass
import concourse.mybir as mybir
import concourse.tile as tile
from concourse import bacc
from concourse.bass_utils import run_bass_kernel_spmd
from concourse.masks import make_identity

B, S, HID, NH, NKV, HD = 1, 2048, 2048, 16, 4, 128
NREP, WIN = 4, 1024
NCORES = 8
HPC = NH // NCORES          # 2 query heads per core
P = 128
SB = S // P                 # 16 seq blocks
IT = 256                    # attention i-tile width (2 query blocks)
NT = S // IT                # 8 i-tiles
WINB = WIN // P             # 8
ST4 = 4                     # 512-wide s-tiles in projections
SW = S // ST4               # 512
INV_SQRT_D = float(1.0 / np.sqrt(HD))

f32 = mybir.dt.float32
bf16 = mybir.dt.bfloat16
MULT = mybir.AluOpType.mult
ADD = mybir.AluOpType.add
EXP = mybir.ActivationFunctionType.Exp

_CACHE = {}


def _build_nc():
    nc = bacc.Bacc(None, target_bir_lowering=False)

    xT = nc.dram_tensor("xT", [HID, S], bf16, kind="ExternalInput")
    wqT = nc.dram_tensor("wqT", [HID, HPC * HD], bf16, kind="ExternalInput")
    wkT = nc.dram_tensor("wkT", [HID, HD], bf16, kind="ExternalInput")
    wvT = nc.dram_tensor("wvT", [HID, HD], bf16, kind="ExternalInput")
    woT = nc.dram_tensor("woT", [HPC * HD, HID], bf16, kind="ExternalInput")
    cosT = nc.dram_tensor("cosT", [HD, S], f32, kind="ExternalInput")
    sinPM = nc.dram_tensor("sinPM", [HD, S], f32, kind="ExternalInput")
    W2 = HPC * IT  # 512: both heads' i-tile side by side
    maskA = nc.dram_tensor("maskA", [P, W2], f32, kind="ExternalInput")
    maskB = nc.dram_tensor("maskB", [P, W2], f32, kind="ExternalInput")
    maskC = nc.dram_tensor("maskC", [P, W2], f32, kind="ExternalInput")
    maskD = nc.dram_tensor("maskD", [P, W2], f32, kind="ExternalInput")
    outT = nc.dram_tensor("outT", [HID, S], f32, kind="ExternalOutput")

    xT_r = xT.rearrange("(c p) s -> p c s", p=P)        # [128, 16, S]
    wqT_r = wqT.rearrange("(c p) m -> p c m", p=P)      # [128, 16, 256]
    wkT_r = wkT.rearrange("(c p) m -> p c m", p=P)
    wvT_r = wvT.rearrange("(c p) m -> p c m", p=P)
    woT_r = woT.rearrange("(c p) s -> p c s", p=P)      # [128, 2, S]
    outT_r = outT.rearrange("(c p) s -> c p s", p=P)    # [16, 128, S]

    HC = HID // P  # 16 contraction chunks

    with tile.TileContext(nc) as tc:
        with tc.tile_pool(name="const", bufs=1) as cpool, \
             tc.tile_pool(name="work", bufs=1) as wpool, \
             tc.tile_pool(name="tmp", bufs=3) as tpool, \
             tc.tile_pool(name="exps", bufs=4) as epool:

            # ---- resident SBUF tensors ----
            x_sb = cpool.tile([P, HC, S], bf16)
            for c in range(HC):
                nc.sync.dma_start(x_sb[:, c, :], xT_r[:, c, :])
            wq_sb = cpool.tile([P, HC, HPC * HD], bf16)
            nc.sync.dma_start(wq_sb[:], wqT_r[:])
            wk_sb = cpool.tile([P, HC, HD], bf16)
            nc.sync.dma_start(wk_sb[:], wkT_r[:])
            wv_sb = cpool.tile([P, HC, HD], bf16)
            nc.sync.dma_start(wv_sb[:], wvT_r[:])
            wo_sb = cpool.tile([P, HPC, S], bf16)
            nc.sync.dma_start(wo_sb[:], woT_r[:])
            cos_sb = cpool.tile([HD, S], f32)
            nc.sync.dma_start(cos_sb[:], cosT[:])
            sin_sb = cpool.tile([HD, S], f32)
            nc.sync.dma_start(sin_sb[:], sinPM[:])
            mA_sb = cpool.tile([P, W2], f32)
            nc.sync.dma_start(mA_sb[:], maskA[:])
            mB_sb = cpool.tile([P, W2], f32)
            nc.sync.dma_start(mB_sb[:], maskB[:])
            mC_sb = cpool.tile([P, W2], f32)
            nc.sync.dma_start(mC_sb[:], maskC[:])
            mD_sb = cpool.tile([P, W2], f32)
            nc.sync.dma_start(mD_sb[:], maskD[:])

            ones_sb = cpool.tile([P, 1], bf16)
            nc.vector.memset(ones_sb[:], 1.0)
            onesr_sb = cpool.tile([1, P], f32)
            nc.vector.memset(onesr_sb[:], 1.0)
            ident_sb = cpool.tile([P, P], bf16)
            make_identity(nc, ident_sb[:])

            # RoPE'd Q^T, head-interleaved per i-tile: [:, t, h*IT:(h+1)*IT]
            qt_sb = wpool.tile([P, NT, W2], bf16)
            kt_sb = wpool.tile([P, S], bf16)         # RoPE'd K^T
            vt_sb = wpool.tile([P, S], bf16)         # V^T (d-major)
            v_sd = wpool.tile([P, SB, HD], bf16)     # V (s-major) for PV lhsT
            ot_sb = wpool.tile([P, HPC, S], bf16)    # attn out^T (normalized)

            # ---- single merged phase: K,V proj -> pipelined Q/attn/out ----
            with tc.tile_pool(name="ps_mm", bufs=2, space="PSUM") as mm_ps, \
                 tc.tile_pool(name="ps_st", bufs=3, space="PSUM") as st_ps, \
                 tc.tile_pool(name="ps_pv", bufs=2, space="PSUM") as pv_ps, \
                 tc.tile_pool(name="ps_on", bufs=1, space="PSUM") as on_ps:

                def rope_epilogue(ps, dsts, st):
                    # dsts: list of (dst_ap, free-slice of the SW window)
                    sl = bass.ts(st, SW)
                    ta = tpool.tile([P, SW], f32, tag="rope_a")
                    nc.vector.tensor_tensor(ta[:], ps[:], cos_sb[:, sl], MULT)
                    tb = tpool.tile([P, SW], f32, tag="rope_b")
                    nc.vector.tensor_tensor(
                        tb[0:64, :], ps[64:128, :], sin_sb[0:64, sl], MULT)
                    nc.vector.tensor_tensor(
                        tb[64:128, :], ps[0:64, :], sin_sb[64:128, sl], MULT)
                    for dst, fsl in dsts:
                        nc.vector.tensor_tensor(dst, ta[:, fsl], tb[:, fsl], ADD)

                def proj(w_sb, mb, st):
                    ps = mm_ps.tile([P, SW], f32, tag="mm512")
                    for c in range(HC):
                        nc.tensor.matmul(
                            ps[:],
                            w_sb[:, c, bass.ts(mb, P)],
                            x_sb[:, c, bass.ts(st, SW)],
                            start=(c == 0), stop=(c == HC - 1),
                        )
                    return ps

                def attend(t):
                    # both heads at once: rhs = [h0 i-tile | h1 i-tile] (512)
                    qb0 = 2 * t
                    jbs = list(range(max(0, qb0 - WINB), qb0 + 2))
                    isl = bass.ts(t, IT)
                    ps_pv = pv_ps.tile([P, W2], f32, tag="pv")
                    ps_on = on_ps.tile([1, W2], f32, tag="on")
                    for idx, jb in enumerate(jbs):
                        ps_st = st_ps.tile([P, W2], f32, tag="st")
                        nc.tensor.matmul(
                            ps_st[:],
                            kt_sb[:, bass.ts(jb, P)],
                            qt_sb[:, t, :],
                            start=True, stop=True,
                        )
                        if jb == qb0 + 1:
                            nc.vector.tensor_tensor(ps_st[:], ps_st[:], mD_sb[:], ADD)
                        elif jb == qb0:
                            nc.vector.tensor_tensor(ps_st[:], ps_st[:], mC_sb[:], ADD)
                        elif qb0 >= WINB and jb == qb0 - WINB:
                            nc.vector.tensor_tensor(ps_st[:], ps_st[:], mA_sb[:], ADD)
                        elif qb0 >= WINB and jb == qb0 - WINB + 1:
                            nc.vector.tensor_tensor(ps_st[:], ps_st[:], mB_sb[:], ADD)
                        e_sb = epool.tile([P, W2], bf16, tag="exp")
                        nc.scalar.activation(e_sb[:], ps_st[:], EXP, scale=INV_SQRT_D)
                        first, last = idx == 0, idx == len(jbs) - 1
                        nc.tensor.matmul(
                            ps_pv[:], v_sd[:, jb, :], e_sb[:],
                            start=first, stop=last)
                        nc.tensor.matmul(
                            ps_on[:], ones_sb[:], e_sb[:],
                            start=first, stop=last)
                    # normalize: ot = pv * (1/sums) broadcast over partitions
                    recip = tpool.tile([1, W2], f32, tag="recip")
                    nc.vector.reciprocal(recip[:], ps_on[:])
                    ps_bc = pv_ps.tile([P, W2], f32, tag="pv")
                    nc.tensor.matmul(
                        ps_bc[:], onesr_sb[:], recip[:], start=True, stop=True)
                    rb = tpool.tile([P, W2], f32, tag="rb")
                    nc.vector.tensor_copy(rb[:], ps_bc[:])
                    for h in range(HPC):
                        hsl = bass.ts(h, IT)
                        nc.vector.tensor_tensor(
                            ot_sb[:, h, isl], ps_pv[:, hsl], rb[:, hsl], MULT)

                def outproj(st):
                    sl = bass.ts(st, SW)
                    for cb in range(HID // P):
                        pso = mm_ps.tile([P, SW], f32, tag="mm512")
                        for mc in range(HPC):
                            nc.tensor.matmul(
                                pso[:],
                                wo_sb[:, mc, bass.ts(cb, P)],
                                ot_sb[:, mc, sl],
                                start=(mc == 0), stop=(mc == HPC - 1),
                            )
                        ob = tpool.tile([P, SW], f32, tag="ob")
                        nc.any.tensor_copy(ob[:], pso[:])
                        nc.sync.dma_start(outT_r[cb, :, sl], ob[:])

                # K, V projections first (shared by both heads / all tiles)
                for st in range(ST4):
                    ps = proj(wk_sb, 0, st)
                    rope_epilogue(ps, [(kt_sb[:, bass.ts(st, SW)], slice(None))], st)
                for st in range(ST4):
                    ps = proj(wv_sb, 0, st)
                    nc.vector.tensor_copy(vt_sb[:, bass.ts(st, SW)], ps[:])
                for sb_i in range(SB):
                    ps_vt = st_ps.tile([P, P], bf16, tag="st")
                    nc.tensor.transpose(
                        ps_vt[:], vt_sb[:, bass.ts(sb_i, P)], ident_sb[:])
                    nc.vector.tensor_copy(v_sd[:, sb_i, :], ps_vt[:])

                # software-pipelined: Q proj for s-tile st, attention/outproj
                # for s-tile st-1 interleave on the PE stream
                for st in range(ST4 + 1):
                    if st < ST4:
                        for head, mb in ((0, 0), (1, 1)):
                            ps = proj(wq_sb, mb, st)
                            dsts = [
                                (qt_sb[:, 2 * st + j, bass.ts(head, IT)],
                                 bass.ts(j, IT))
                                for j in range(SW // IT)
                            ]
                            rope_epilogue(ps, dsts, st)
                    if st > 0:
                        attend(2 * (st - 1))
                        attend(2 * st - 1)
                        outproj(st - 1)

    nc.compile()
    return nc


def _host_inputs(x, attention_mask, wq, wk, wv, wo):
    """Build the 8 per-core input maps from full inputs."""
    bf = ml_dtypes.bfloat16
    x2 = np.ascontiguousarray(np.asarray(x, dtype=np.float32).reshape(S, HID))
    xT_np = np.ascontiguousarray(x2.T.astype(bf))

    inv = 1.0 / (10000.0 ** (np.arange(0, HD, 2, dtype=np.float32) / HD))
    freqs = np.outer(inv, np.arange(S, dtype=np.float32))      # [64, S]
    cosT_np = np.ascontiguousarray(
        np.concatenate([np.cos(freqs)] * 2, 0).astype(np.float32))
    sinF = np.sin(freqs)
    sinPM_np = np.ascontiguousarray(
        np.concatenate([-sinF, sinF], 0).astype(np.float32))

    am2 = np.asarray(attention_mask, dtype=np.float32).reshape(S, S)
    mC1 = am2[0:P, 0:P].T                     # diag block (St coords)
    mB1 = am2[WIN:WIN + P, 0:P].T             # window tail block
    mF1 = am2[WIN + P:WIN + 2 * P, 0:P].T     # fully masked block
    z = np.zeros((P, P), np.float32)
    # 512-wide masks for the head-batched [h0 i-tile | h1 i-tile] layout
    mA = np.ascontiguousarray(np.concatenate([mB1, mF1, mB1, mF1], 1))
    mB = np.ascontiguousarray(np.concatenate([z, mB1, z, mB1], 1))
    mC = np.ascontiguousarray(np.concatenate([mC1, z, mC1, z], 1))
    mD = np.ascontiguousarray(np.concatenate([mF1, mC1, mF1, mC1], 1))

    wq2 = np.asarray(wq, dtype=np.float32)
    wk2 = np.asarray(wk, dtype=np.float32)
    wv2 = np.asarray(wv, dtype=np.float32)
    wo2 = np.asarray(wo, dtype=np.float32)

    in_maps = []
    for d in range(NCORES):
        g = d // 2
        in_maps.append({
            "xT": xT_np,
            "wqT": np.ascontiguousarray(
                wq2[HPC * HD * d:HPC * HD * (d + 1), :].T.astype(bf)),
            "wkT": np.ascontiguousarray(
                wk2[HD * g:HD * (g + 1), :].T.astype(bf)),
            "wvT": np.ascontiguousarray(
                wv2[HD * g:HD * (g + 1), :].T.astype(bf)),
            "woT": np.ascontiguousarray(
                wo2[:, HPC * HD * d:HPC * HD * (d + 1)].T.astype(bf)),
            "cosT": cosT_np,
            "sinPM": sinPM_np,
            "maskA": mA, "maskB": mB, "maskC": mC, "maskD": mD,
        })
    return in_maps


def run(inputs, trace=False):
    if "nc" not in _CACHE:
        _CACHE["nc"] = _build_nc()
    nc = _CACHE["nc"]
    in_maps = _host_inputs(**inputs)
    res = run_bass_kernel_spmd(
        nc, in_maps, core_ids=list(range(NCORES)), trace=trace)
    acc = np.zeros((HID, S), np.float64)
    for d in range(NCORES):
        acc += np.asarray(res.results[d]["outT"], dtype=np.float64)
    out = acc.T.astype(np.float32).reshape(B, S, HID)
    return out, res.exec_time_ns


def kernel(**inputs) -> np.ndarray:
    out, _ = run(inputs, trace=False)
    return out


def bench(inputs, iters=101):
    """Time the NEFF on silicon: chain `iters` executions inside one XLA
    program (serialized by feeding iter i's outputs as iter i+1's donated
    output buffers), subtract the 1-iteration program's wall time, divide.
    Returns (outputs_of_first_iter_as_full_result, exec_ns_estimate)."""
    import time
    import jax
    from jax.experimental.shard_map import shard_map
    from jax.sharding import Mesh, NamedSharding, PartitionSpec
    from concourse import mybir as _mybir
    from concourse.bass2jax import (
        _bass_exec_p, install_neuronx_cc_hook, partition_id_tensor)

    if "nc" not in _CACHE:
        _CACHE["nc"] = _build_nc()
    nc = _CACHE["nc"]
    install_neuronx_cc_hook()
    in_maps = _host_inputs(**inputs)

    partition_name = (
        nc.partition_id_tensor.name if nc.partition_id_tensor else None)
    in_names, out_names, out_avals, zero_outs = [], [], [], []
    for alloc in nc.m.functions[0].allocations:
        if not isinstance(alloc, _mybir.MemoryLocationSet):
            continue
        name = alloc.memorylocations[0].name
        if alloc.kind == "ExternalInput":
            if name != partition_name:
                in_names.append(name)
        elif alloc.kind == "ExternalOutput":
            out_names.append(name)
            shape = tuple(alloc.tensor_shape)
            dtype = _mybir.dt.np(alloc.dtype)
            out_avals.append(jax.core.ShapedArray(shape, dtype))
            zero_outs.append(np.zeros(shape, dtype))
    n_params = len(in_names)
    all_names = list(in_names) + list(out_names)
    if partition_name is not None:
        all_names.append(partition_name)

    def _make_body(k):
        def _body(*args):
            ins = list(args[:n_params])
            cur = list(args[n_params:])
            for _ in range(k):
                operands = [*ins, *cur]
                if partition_name is not None:
                    operands.append(partition_id_tensor())
                outs = _bass_exec_p.bind(
                    *operands,
                    out_avals=tuple(out_avals),
                    in_names=tuple(all_names),
                    out_names=tuple(out_names),
                    lowering_input_output_aliases=(),
                    sim_require_finite=True,
                    sim_require_nnan=True,
                    nc=nc,
                )
                cur = list(outs)
            return tuple(cur)
        return _body

    devices = jax.devices()[:NCORES]
    mesh = Mesh(np.asarray(devices), ("core",))
    spec = PartitionSpec("core")
    nin = n_params + len(out_names)
    concat_in = [
        np.concatenate([np.asarray(in_maps[c][n]) for c in range(NCORES)], axis=0)
        for n in in_names
    ] + [np.zeros((NCORES * z.shape[0], *z.shape[1:]), z.dtype) for z in zero_outs]
    sh = NamedSharding(mesh, spec)
    dev_in = [jax.device_put(a, sh) for a in concat_in]

    fn = jax.jit(
        shard_map(_make_body(1), mesh=mesh,
                  in_specs=(spec,) * nin, out_specs=(spec,) * len(out_names),
                  check_rep=False),
        keep_unused=True,
    )

    # warmup + correctness capture
    outs1 = fn(*dev_in)
    jax.block_until_ready(outs1)
    first = [
        {n: np.asarray(outs1[i]).reshape(NCORES, *out_avals[i].shape)[c]
         for i, n in enumerate(out_names)}
        for c in range(NCORES)
    ]
    fn(*dev_in)[0].block_until_ready()

    def _time(m, reps=4):
        # m async dispatches pipelined on the device queue, one block
        best = float("inf")
        for _ in range(reps):
            t0 = time.perf_counter()
            rs = [fn(*dev_in) for _ in range(m)]
            jax.block_until_ready(rs)
            best = min(best, time.perf_counter() - t0)
        return best

    t1, tk = _time(1), _time(iters)
    exec_ns = (tk - t1) / (iters - 1) * 1e9

    acc = np.zeros((HID, S), np.float64)
    for c in range(NCORES):
        acc += first[c]["outT"].astype(np.float64)
    out = acc.T.astype(np.float32).reshape(B, S, HID)
    return out, exec_ns, t1 * 1e9, tk * 1e9

